# revision 6
# baseline (speedup 1.0000x reference)
"""Trainium2 Bass kernel for nn_Attention_28724741275862.

Reference computation (per batch b):
    dec_part[i,o] = dec[b] @ W_dec.T          # [64, 512]
    enc_part[j,o] = enc[b] @ W_enc.T          # [512, 512]
    logits[i,j,o] = dec_part[i,o] + enc_part[j,o] + bias[o]
    alpha = log_softmax(logits, axis=o)
    ctx[i,o] = sum_j alpha[i,j,o] * enc[b][j,o]

Factorization used here (exact in fp32, ~2e-3 rel err with bf16 operands):
    LSE[i,j] = log(sum_o exp(dec_part[i,o]) * exp(enc_part[j,o] + bias[o]))
             = log( (E_enc @ E_dec)[j,i] )            # a matmul over o!
    ctx[i,o] = dec_part[i,o]*S_enc[o] + C'[o] - (LSE @ enc[b])[i,o]
      S_enc[o] = sum_j enc[j,o]
      C'[o]    = sum_j (enc_part[j,o] + bias[o]) * enc[j,o]
               = C0'[o] + bias[o]*S_enc[o]

So the O(T_dec*T_enc*H2) log-softmax collapses into 4 matmuls + cheap
elementwise work. Sharding: data-parallel over batch B=8 across the 8
cores (encoderOutput/decoderInput sharded on dim 0, W/b replicated); no
collectives. Everything on-chip is computed in transposed layout
[feature_on_partitions, i_free] so per-feature broadcasts are
per-partition scalar operands.

Self-contained: hardcodes shapes B=8, T_dec=64, T_enc=512, H2=512.
"""

import sys

for _p in ("/opt/trn_rl_repo",):
    if _p not in sys.path:
        sys.path.insert(0, _p)

import numpy as np
import ml_dtypes

import concourse.bass as bass
import concourse.tile as tile
from concourse import bacc, mybir
from concourse.bass_utils import run_bass_kernel_spmd

B, T_DEC, T_ENC, H2 = 8, 64, 512, 512
P = 128  # SBUF partitions
NB = H2 // P  # 4 feature blocks

BF16 = mybir.dt.bfloat16
F32 = mybir.dt.float32
AF = mybir.ActivationFunctionType
ALU = mybir.AluOpType

_CACHE = {}


def _build_nc():
    nc = bacc.Bacc(None, target_bir_lowering=False)

    # Per-core DRAM inputs (core = batch):
    #  encN [T_enc, H2] bf16   natural enc       (lhsT for ctx2 matmul)
    #  encT [H2, T_enc] bf16   transposed enc    (rhs for enc_part, C' term)
    #  decT [H2, T_dec] bf16   transposed dec    (rhs for dec_part)
    #  WT   [2*H2, H2]  bf16   W transposed; rows 0:H2 = W_dec^T, H2: = W_enc^T
    #  b4   [P, NB]     f32    bias swizzled per-partition: b4[p,k] = b[k*P+p]
    encN = nc.dram_tensor("encN", [T_ENC, H2], BF16, kind="ExternalInput")
    encT = nc.dram_tensor("encT", [H2, T_ENC], BF16, kind="ExternalInput")
    decT = nc.dram_tensor("decT", [H2, T_DEC], BF16, kind="ExternalInput")
    WT = nc.dram_tensor("WT", [2 * H2, H2], BF16, kind="ExternalInput")
    b4 = nc.dram_tensor("b4", [P, NB], F32, kind="ExternalInput")
    # out = ctx^T [H2, T_dec] f32; host transposes back.
    out = nc.dram_tensor("out", [H2, T_DEC], F32, kind="ExternalOutput")

    encN_r = encN[:, :].rearrange("(a p) o -> p a o", p=P)
    encT_r = encT[:, :].rearrange("(a p) j -> p a j", p=P)
    decT_r = decT[:, :].rearrange("(a p) i -> p a i", p=P)
    WT_r = WT[:, :].rearrange("(a p) o -> p a o", p=P)
    out_r = out[:, :].rearrange("(a p) i -> p a i", p=P)

    with tile.TileContext(nc) as tc:
        with (
            tc.tile_pool(name="ins", bufs=1) as ins,
            tc.tile_pool(name="mids", bufs=1) as mids,
            tc.tile_pool(name="ppool", bufs=1, space="PSUM") as ppool,
            tc.tile_pool(name="spool", bufs=1, space="PSUM") as spool,
        ):
            # ---- input DMAs (HWDGE) ----
            wte_t = [ins.tile([P, H2], BF16, name=f"wte{d}", tag=f"wte{d}") for d in range(NB)]
            eT_t = [ins.tile([P, T_ENC], BF16, name=f"eT{d}", tag=f"eT{d}") for d in range(NB)]
            for db in range(NB):
                nc.sync.dma_start(out=wte_t[db][:, :], in_=WT_r[:, NB + db, :])
                nc.sync.dma_start(out=eT_t[db][:, :], in_=encT_r[:, db, :])
            wtd_t = ins.tile([P, NB, H2], BF16)
            nc.sync.dma_start(out=wtd_t[:, :, :], in_=WT_r[:, 0:NB, :])
            dT_t = ins.tile([P, NB, T_DEC], BF16)
            nc.sync.dma_start(out=dT_t[:, :, :], in_=decT_r[:, :, :])
            b4_t = ins.tile([P, NB], F32)
            nc.sync.dma_start(out=b4_t[:, :], in_=b4[:, :])
            eN_t = ins.tile([P, NB, H2], BF16)
            nc.sync.dma_start(out=eN_t[:, :, :], in_=encN_r[:, :, :])

            # ---- A1: enc_part^T[o, j] += W_enc^T[d, o].T @ enc^T[d, j] ----
            pp = [ppool.tile([P, T_ENC], F32, name=f"pp{o}", tag=f"pp{o}") for o in range(NB)]
            for db in range(NB):
                for ob in range(NB):
                    nc.tensor.matmul(
                        pp[ob][:, :],
                        lhsT=wte_t[db][:, ob * P : (ob + 1) * P],
                        rhs=eT_t[db][:, :],
                        start=(db == 0),
                        stop=(db == NB - 1),
                    )

            # ---- A2: dec_part^T[o, i] += W_dec^T[d, o].T @ dec^T[d, i] ----
            pd = spool.tile([P, NB, T_DEC], F32, name="pdall")
            for ob in range(NB):
                for db in range(NB):
                    nc.tensor.matmul(
                        pd[:, ob, :],
                        lhsT=wtd_t[:, db, ob * P : (ob + 1) * P],
                        rhs=dT_t[:, db, :],
                        start=(db == 0),
                        stop=(db == NB - 1),
                    )

            # ---- B: exponentials (ACT), keep dec_part, C' partial (DVE) ----
            ee_t = [mids.tile([P, T_ENC], BF16, name=f"ee{o}", tag=f"ee{o}") for o in range(NB)]
            ed_t = [mids.tile([P, T_DEC], BF16, name=f"ed{o}", tag=f"ed{o}") for o in range(NB)]
            dp_t = [mids.tile([P, T_DEC], F32, name=f"dp{o}", tag=f"dp{o}") for o in range(NB)]
            cp_t = mids.tile([P, NB], F32)  # C0' per feature block
            junk = mids.tile([P, T_ENC], F32)  # ttr elementwise product sink
            for ob in range(NB):
                # E_enc^T = exp(enc_part^T + bias)
                nc.scalar.activation(
                    ee_t[ob][:, :],
                    pp[ob][:, :],
                    AF.Exp,
                    bias=b4_t[:, ob : ob + 1],
                )
                # E_dec^T = exp(dec_part^T)
                nc.scalar.activation(ed_t[ob][:, :], pd[:, ob, :], AF.Exp)
                # keep dec_part^T for the final combine
                nc.vector.tensor_copy(dp_t[ob][:, :], pd[:, ob, :])
                # C0'[o] = sum_j enc_part^T[o,j] * enc^T[o,j]
                # (tensor_tensor_reduce NEFFs fail at runtime here; use
                # separate mult + reduce)
                nc.vector.tensor_tensor(
                    out=junk[:, :],
                    in0=pp[ob][:, :],
                    in1=eT_t[ob][:, :],
                    op=ALU.mult,
                )
                nc.vector.reduce_sum(
                    out=cp_t[:, ob : ob + 1],
                    in_=junk[:, :],
                    axis=mybir.AxisListType.X,
                )

            # ---- C: S^T[j, i] += E_enc^T[o, j].T @ E_dec^T[o, i] ----
            ps = spool.tile([P, NB, T_DEC], F32, name="psall")
            for jb in range(NB):
                for ob in range(NB):
                    nc.tensor.matmul(
                        ps[:, jb, :],
                        lhsT=ee_t[ob][:, jb * P : (jb + 1) * P],
                        rhs=ed_t[ob][:, :],
                        start=(ob == 0),
                        stop=(ob == NB - 1),
                    )

            # ---- D: LSE^T = ln(S^T), with a ones column for S_enc ----
            lt_t = [mids.tile([P, T_DEC + 1], BF16, name=f"lt{j}", tag=f"lt{j}") for j in range(NB)]
            for jb in range(NB):
                nc.vector.memset(lt_t[jb][:, T_DEC : T_DEC + 1], 1.0)
                nc.scalar.activation(lt_t[jb][:, 0:T_DEC], ps[:, jb, :], AF.Ln)

            # ---- E: [ctx2^T | S_enc][o, :] += enc[j, o].T @ [LSE^T | 1] ----
            pc = spool.tile([P, NB, T_DEC + 1], F32, name="pcall")
            for ob in range(NB):
                for jb in range(NB):
                    nc.tensor.matmul(
                        pc[:, ob, :],
                        lhsT=eN_t[:, jb, ob * P : (ob + 1) * P],
                        rhs=lt_t[jb][:, :],
                        start=(jb == 0),
                        stop=(jb == NB - 1),
                    )

            # ---- G: ctx^T = dec_part^T * S_enc + (b*S_enc + C0') - ctx2^T ----
            se_t = mids.tile([P, NB], F32)
            fix_t = mids.tile([P, NB], F32)
            ctxo = mids.tile([P, NB, T_DEC], F32)
            for ob in range(NB):
                nc.vector.tensor_copy(se_t[:, ob : ob + 1], pc[:, ob, T_DEC : T_DEC + 1])
                # fix = b*S_enc + C0'
                nc.vector.tensor_scalar(
                    out=fix_t[:, ob : ob + 1],
                    in0=se_t[:, ob : ob + 1],
                    scalar1=b4_t[:, ob : ob + 1],
                    scalar2=cp_t[:, ob : ob + 1],
                    op0=ALU.mult,
                    op1=ALU.add,
                )
                # ctx = dp*S_enc + fix
                nc.vector.tensor_scalar(
                    out=ctxo[:, ob, :],
                    in0=dp_t[ob][:, :],
                    scalar1=se_t[:, ob : ob + 1],
                    scalar2=fix_t[:, ob : ob + 1],
                    op0=ALU.mult,
                    op1=ALU.add,
                )
                # ctx -= ctx2
                nc.vector.tensor_tensor(
                    out=ctxo[:, ob, :],
                    in0=ctxo[:, ob, :],
                    in1=pc[:, ob, 0:T_DEC],
                    op=ALU.subtract,
                )
            nc.sync.dma_start(out=out_r[:, :, :], in_=ctxo[:, :, :])

    nc.finalize()
    return nc


def _prep_in_maps(encoderOutput, decoderInput, W, b):
    bf = ml_dtypes.bfloat16
    WT = np.ascontiguousarray(W.T).astype(bf)
    b4 = np.ascontiguousarray(np.asarray(b, np.float32).reshape(NB, P).T)
    in_maps = []
    for core in range(B):
        e = np.asarray(encoderOutput[core], np.float32)
        d = np.asarray(decoderInput[core], np.float32)
        in_maps.append(
            {
                "encN": e.astype(bf),
                "encT": np.ascontiguousarray(e.T).astype(bf),
                "decT": np.ascontiguousarray(d.T).astype(bf),
                "WT": WT,
                "b4": b4,
            }
        )
    return in_maps


def kernel(encoderOutput, decoderInput, W, b, _trace=False):
    if "nc" not in _CACHE:
        _CACHE["nc"] = _build_nc()
    nc = _CACHE["nc"]
    in_maps = _prep_in_maps(encoderOutput, decoderInput, W, b)
    res = run_bass_kernel_spmd(nc, in_maps, core_ids=list(range(B)), trace=_trace)
    outs = np.stack([np.asarray(r["out"], np.float32).T for r in res.results])
    if _trace:
        _CACHE["last_result"] = res
    return outs
